# revision 36
# baseline (speedup 1.0000x reference)
"""Trainium2 Bass kernel for the quantized BasicBlock (conv3x3/s2 + fakequant + conv3x3/s1 + fakequant).

Sharding: data-parallel over batch across 8 cores (8 images each), weights replicated.

Device math (per core, B=8):
  conv1: implicit GEMM, 9 taps x 2 ci-blocks, single-pass fp16 (x rounded to fp16,
         ~11-bit mantissa; the act1 integer rounding absorbs the error well within
         the 2e-2 gate), integer-valued fp16 weights, fp32 PSUM accum.
  act1:  v = P1*(s_w1/s_a1) + bq1/s_a1; y = clip(rne(v), -128, 127) via the fp32
         magic-number trick on the DVE; y stored as integer-valued bf16 into a
         zero-padded [16x16] layout for conv2.
  conv2: 1-D Winograd F(2,3) along the column axis: V = B^T d (exact ints in
         bf16, 4 DVE ops per act chunk), U = G w (exact half-ints in fp16, host),
         GEMM over (ky, m) in fp32 PSUM (exact), inverse A^T on the DVE (exact
         ints) -> 1.5x fewer PE rows than direct.
  act2:  v2 = P2*(s_a1*s_w2/s_a2) + bq2/s_a2; out = clip(rne(v2), -128, 127) * s_a2.

Stride-2 conv1 is handled by a host-side phase split: x is scattered into 2x2 parity
planes zero-padded to 15x16 rows (32B-aligned rows); 6 plane variants (normal +
one-column-shifted) make every tap window start 4-byte aligned — misaligned rhs
windows cost ~15% per matmul on TRN2.

Input DMAs are chunked in first-use order on the Sync HWDGE queue while ~58 junk
warm-up matmuls cover the load latency and ramp the PE clock; outputs stream on
the Scalar queue.
"""
import os
import sys
from contextlib import ExitStack

import numpy as np
import ml_dtypes

for _p in ("/opt/trn_rl_repo",):
    if _p not in sys.path and os.path.isdir(_p):
        sys.path.insert(0, _p)

import concourse.bacc as bacc
import concourse.tile as tile
import concourse.mybir as mybir
from concourse.bass_utils import run_bass_kernel_spmd

BF16 = ml_dtypes.bfloat16
N_CORES = 8
B_PER = 8           # images per core
MAGIC = float(np.float32(1.5 * 2 ** 23))   # fp32 RNE rounding magic
Alu = mybir.AluOpType
dt = mybir.dt

# tap index k in {0,1,2} -> (parity s, window start offset) for the phase planes
_TAP = {0: (1, 0), 1: (0, 1), 2: (1, 1)}

# cb0 tap order, chosen so plane-variant demand follows DMA arrival order
TAP_ORDER = [0, 6, 3, 1, 7, 2, 8, 4, 5]
# (plane, col_offset) -> x_d variant index; 's' variants are pre-shifted one
# column left on the host so every window starts at column 0 (4B-aligned)
_PVAR = {(3, 0): 0, (1, 0): 1, (2, 1): 2, (3, 1): 3, (0, 1): 4, (1, 1): 5}
# w1's tap axis is permuted into TAP_ORDER on the host, so device-side w1
# indexing uses the order position
_TAP_POS = {t9: o for o, t9 in enumerate(TAP_ORDER)}


def _phase_planes(x):
    """(B, C, 28, 28) f32 -> (B, C, 2, 2, 15, 16): plane[sr][sc][q+1][p+1] = x[2q+sr][2p+sc].

    Rows are padded to 16 so SBUF row stride is 32 B (aligned); col 15 is
    never read by any tap window."""
    B, C = x.shape[:2]
    out = np.zeros((B, C, 2, 2, 15, 16), np.float32)
    for sr in (0, 1):
        for sc in (0, 1):
            out[:, :, sr, sc, 1:15, 1:15] = x[:, :, sr::2, sc::2]
    return out


def _quant_weights(w):
    """Per-tensor int8 narrow-range fake quant; returns (int-valued f32 weights, scale)."""
    s = np.float32(np.max(np.abs(w))) / np.float32(127.0)
    wq = np.clip(np.round(w / s), -127, 127).astype(np.float32)
    return wq, s


def _w2_wino(w_int):
    """(512co, 512ci, 3, 3) ints -> (ci_blk 4, 128, ky 3, m 4, cb 4, co 128) fp16.

    1-D Winograd F(2,3) weight transform along kx: U0 = g0, U1 = (g0+g1+g2)/2,
    U2 = (g0-g1+g2)/2, U3 = g2 — half-integers <= 190.5, exact in fp16."""
    g0 = w_int[..., 0]
    g1 = w_int[..., 1]
    g2 = w_int[..., 2]                                    # (co, ci, ky)
    U = np.stack([g0, (g0 + g1 + g2) * 0.5, (g0 - g1 + g2) * 0.5, g2], axis=0)
    t = U.transpose(2, 3, 0, 1)                           # (ci, ky, m, co)
    t = t.reshape(4, 128, 3, 4, 4, 128)                   # (ci_blk, ci, ky, m, cb, co)
    return np.ascontiguousarray(t).astype(np.float16)


def _w_lhsT(w_int, n_ci_blk):
    """(Cout=512, Cin, 3, 3) int-valued -> (ci_blk, 128, 9, 4, 128) bf16 stationary layout."""
    t = w_int.transpose(2, 3, 1, 0)                      # (3, 3, Cin, 512)
    t = t.reshape(9, n_ci_blk, 128, 4, 128)              # (tap, ci_blk, ci_p, co_blk, co)
    return np.ascontiguousarray(t.transpose(1, 2, 0, 3, 4)).astype(BF16)


_skip_ldw = [False]
_orig_InstMatmult = mybir.InstMatmult


def _patched_InstMatmult(*a, **kw):
    if _skip_ldw[0]:
        kw.setdefault("ldweights", False)
    return _orig_InstMatmult(*a, **kw)


def build_program(scale1, scale2, out_scale):
    """Build the (per-core SPMD) Bass program with the given fp32 immediates."""
    nc = bacc.Bacc("TRN2", target_bir_lowering=False, debug=False,
                   num_devices=N_CORES)

    mybir.InstMatmult = _patched_InstMatmult
    try:
        return _build_body(nc, scale1, scale2, out_scale)
    finally:
        mybir.InstMatmult = _orig_InstMatmult


def _build_body(nc, scale1, scale2, out_scale):
    NT = 4

    # 6 plane variants (normal / col-shifted) so every conv1 tap window starts
    # 4-byte aligned; order = DMA arrival order = cb0 tap demand order.
    x_d = nc.dram_tensor("x16", (128, 2, 6, B_PER, 15, 16), dt.float16, kind="ExternalInput")
    w1_d = nc.dram_tensor("w1", (2, 128, 9, 4, 128), dt.int8, kind="ExternalInput")
    w2_d = nc.dram_tensor("w2", (4, 128, 3, 4, 4, 128), dt.float16, kind="ExternalInput")
    b1_d = nc.dram_tensor("b1", (128, 4), dt.float32, kind="ExternalInput")
    b2_d = nc.dram_tensor("b2", (128, 4), dt.float32, kind="ExternalInput")
    out_d = nc.dram_tensor("out", (512, B_PER, 2, 14, 7), dt.float32, kind="ExternalOutput")

    def mm(out_ap, w_ap, rhs, start, stop, reuse):
        # reuse=True -> PE keeps the already-loaded stationary weights
        _skip_ldw[0] = reuse
        try:
            nc.tensor.matmul(out_ap, w_ap, rhs, start=start, stop=stop)
        finally:
            _skip_ldw[0] = False

    with tile.TileContext(nc) as tc, ExitStack() as ctx:
        const = ctx.enter_context(tc.tile_pool(name="const", bufs=1))
        psum = ctx.enter_context(tc.tile_pool(name="psum", bufs=8, space="PSUM"))
        tmp = ctx.enter_context(tc.tile_pool(name="tmp", bufs=2))
        outp = ctx.enter_context(tc.tile_pool(name="outp", bufs=2))

        # --- SBUF allocations: one tile per DMA chunk for fine-grained deps ---
        # x plane variants: [v][b] -> [128, n, 15, 16]
        x_t = [const.tile([128, 2, B_PER, 15, 16], dt.float16, tag=f"xh{v}", name=f"xh{v}")
               for v in range(6)]
        # w1: [b] -> [128, tap, co_blk, co]; weights arrive int8, DVE converts
        # them to fp16 (values are integers in [-127,127], exact either way)
        w1_t = [const.tile([128, 9, 4, 128], dt.float16, tag=f"w1{b}", name=f"w1t{b}") for b in range(2)]
        w2_t = [const.tile([128, 3, 4, 4, 128], dt.float16, tag=f"w2{b}", name=f"w2t{b}") for b in range(4)]
        w1i_t = [const.tile([128, 9, 4, 128], dt.int8, tag=f"w1i{b}", name=f"w1i{b}") for b in range(2)]
        b1_t = const.tile([128, 4], dt.float32, tag="b1")
        b2_t = const.tile([128, 4], dt.float32, tag="b2")
        act_t = const.tile([128, 4, B_PER, 16, 16], dt.bfloat16, tag="act")  # padded act1
        # Winograd-domain act: V[m][cb][img][row][tile], tile dim padded to 8
        # so every conv2 rhs window starts 16B-aligned
        v_t = const.tile([128, 4, 4, B_PER, 16, 8], dt.bfloat16, tag="vt")
        wz = const.tile([128, 256], dt.bfloat16, tag="wz")

        # PE warm-up source zeros; act pad memsets go on the DVE *after* the
        # w1 casts (emitted below) — gpsimd memsets are slow and their SBUF
        # traffic stalls the startup casts, while the DVE is idle from cast
        # end (~20us) until the first act1 epilogue (~29us).
        nc.vector.memset(wz[:], 0.0)

        # --- input loads in first-use order on the Sync HWDGE queue (the two
        # HWDGE queues share HBM bandwidth, so splitting input across both
        # gains nothing; outputs use the Scalar queue) ---
        def load(dst, src):
            nc.sync.dma_start(out=dst, in_=src)

        def load_plane(v):
            load(x_t[v][:], x_d[:, :, v])

        # w1 first (small); tap axis is TAP_ORDER-permuted on host and the
        # int8->fp16 casts are chunked per (tap, ci_blk) in demand order.
        # The first two taps ship as their own small chunk so their DMA
        # completion semaphore fires early and the first casts start sooner.
        for b in range(2):
            load(w1i_t[b][:, 0:2], w1_d[b][:, 0:2])
        for b in range(2):
            load(w1i_t[b][:, 2:9], w1_d[b][:, 2:9])
        for o in range(9):
            for b in range(2):
                nc.vector.tensor_copy(w1_t[b][:, o], w1i_t[b][:, o])
        nc.vector.memset(act_t[:], 0.0)
        load_plane(0)
        load(b1_t[:], b1_d[:])
        for v in range(1, 6):
            load_plane(v)
        for b in range(4):
            load(w2_t[b][:], w2_d[b])
        load(b2_t[:], b2_d[:])

        def quant_chain(dst, src, sc, bias_ap, width=392):
            """dst = clip(rne(src*sc + bias), -128, 127) on the DVE (3 fused ops)."""
            tt = tmp.tile([128, width], dt.float32, tag=f"tt{width}", name="tt")
            nc.vector.tensor_scalar(tt[:], src, sc, bias_ap, op0=Alu.mult, op1=Alu.add)
            nc.vector.tensor_scalar(tt[:], tt[:], MAGIC, MAGIC + 127.0, op0=Alu.add, op1=Alu.min)
            nc.vector.tensor_scalar(dst, tt[:], MAGIC - 128.0, -MAGIC, op0=Alu.max, op1=Alu.add)
            return tt

        # PE warm-up: junk matmuls on the zeroed tile during the input-DMA wait
        # so the HAM clock gate is at full rate when the real stream starts.
        wps = psum.tile([128, 512], dt.float32, tag="ps", name="warmps")
        for i in range(50):
            nc.tensor.matmul(wps[:, 0:256], wz[:, 0:128], wz[:, 0:256],
                             start=True, stop=True)

        # --- conv1 + act1 ---
        # cb0 is tap-major: plane demand spread over the whole 144-MM group to
        # match DMA delivery. cb1-3 are nt-major: each psum bank finishes early
        # and its epilogue overlaps the remaining banks' matmuls.
        def conv1_group(cb, t9, b, ps_list, nts):
            # one stationary weight (t9, b, cb) serving len(nts) matmuls;
            # only the first self-loads the PE array
            ky, kx = divmod(t9, 3)
            sr, r0 = _TAP[ky]
            sc_, c0 = _TAP[kx]
            v = _PVAR[(sr * 2 + sc_, c0)]
            w_ap = w1_t[b][:, _TAP_POS[t9], cb, :]
            for i, nt in enumerate(nts):
                rhs = x_t[v][:, b, 2 * nt:2 * nt + 2, r0:r0 + 14, 0:14]
                mm(ps_list[i][:, 0:392], w_ap, rhs,
                   start=(t9 == TAP_ORDER[0] and b == 0),
                   stop=(t9 == TAP_ORDER[-1] and b == 1),
                   reuse=i > 0)

        def act1_chunk(cb, nt, ps):
            quant_chain(act_t[:, cb, 2 * nt:2 * nt + 2, 1:15, 1:15],
                        ps[:, 0:392], scale1, b1_t[:, cb:cb + 1])
            # 1-D Winograd data transform V = B^T d over the column axis:
            # V0 = d0-d2, V1 = d1+d2, V2 = d2-d1, V3 = d1-d3 (per 4-col tile,
            # stride 2; all 16 rows incl. pads; exact small ints in bf16)
            a = act_t[:, cb, 2 * nt:2 * nt + 2, :, :]
            ev0 = a[:, :, :, 0:13:2]
            od1 = a[:, :, :, 1:14:2]
            ev2 = a[:, :, :, 2:15:2]
            od3 = a[:, :, :, 3:16:2]
            dst = lambda m: v_t[:, m, cb, 2 * nt:2 * nt + 2, :, 0:7]
            nc.vector.tensor_tensor(dst(0), ev0, ev2, op=Alu.subtract)
            nc.vector.tensor_tensor(dst(1), od1, ev2, op=Alu.add)
            nc.vector.tensor_tensor(dst(2), ev2, od1, op=Alu.subtract)
            nc.vector.tensor_tensor(dst(3), od1, od3, op=Alu.subtract)

        for cb in range(4):
            if cb == 0:
                # tap-major: plane demand spread over the whole group to match
                # the DMA delivery ramp; 8 matmuls per weight load
                ps_n = [psum.tile([128, 512], dt.float32, tag="ps", name="ps")
                        for _ in range(NT)]
                for t9 in TAP_ORDER:
                    for b in range(2):
                        conv1_group(cb, t9, b, ps_n, range(NT))
                for nt in range(NT):
                    act1_chunk(cb, nt, ps_n[nt])
            else:
                # nt-pair-major: each bank pair finishes at half-time so its
                # epilogue overlaps the rest; the last cb runs single-nt
                # groups so its final epilogue chain is short (conv2's first
                # taps wait on it)
                halves = ([[0, 1], [2, 3]] if cb < 3 else [[0], [1], [2], [3]])
                for nts in halves:
                    ps_p = [psum.tile([128, 512], dt.float32, tag="ps", name="ps")
                            for _ in nts]
                    for t9 in TAP_ORDER:
                        for b in range(2):
                            conv1_group(cb, t9, b, ps_p, nts)
                    for i, nt in enumerate(nts):
                        act1_chunk(cb, nt, ps_p[i])

        # --- conv2 (1-D Winograd) + act2 ---
        # For each (out-cb, image-pair chunk): 8 PSUM banks hold the 4 m-
        # positions x 2 chunks; GEMM accumulates over (ky, ci-blk). The
        # epilogue applies the inverse transform A^T (o0 = m0+m1+m2,
        # o1 = m1-m2-m3, both exact ints in fp32), then the act2 quant chain.
        def conv2_epilogue(cb, i0, ni, psm):
            # psm: list of 4 PSUM tiles [128, ni*98] (m = 0..3) for images
            # i0..i0+ni (ni <= 4). Only one PSUM operand is allowed per DVE
            # op, so m1 is staged to SBUF first.
            w = ni * 98
            tq = tmp.tile([128, 4, 2, 14, 7], dt.float32, tag="tq", name="tq")
            s1 = tmp.tile([128, 392], dt.float32, tag="s1", name="s1")
            ti = tmp.tile([128, 392], dt.float32, tag="ti", name="ti")
            t2 = tmp.tile([128, 392], dt.float32, tag="t2", name="t2")
            nc.vector.tensor_copy(s1[:, 0:w], psm[1][:, 0:w])
            nc.vector.tensor_tensor(ti[:, 0:w], psm[0][:, 0:w], s1[:, 0:w], op=Alu.add)
            nc.vector.tensor_tensor(tq[:, 0:ni, 0], ti[:, 0:w], psm[2][:, 0:w], op=Alu.add)
            nc.vector.tensor_tensor(t2[:, 0:w], s1[:, 0:w], psm[2][:, 0:w], op=Alu.subtract)
            nc.vector.tensor_tensor(tq[:, 0:ni, 1], t2[:, 0:w], psm[3][:, 0:w], op=Alu.subtract)
            ot = outp.tile([128, 784], dt.float32, tag="ot", name="ot")
            tq_dst = tmp.tile([128, 784], dt.float32, tag="tq2", name="tq2")
            quant_chain(tq_dst[:, 0:2 * w], tq[:, 0:ni], scale2, b2_t[:, cb:cb + 1],
                        width=2 * w)
            nc.vector.tensor_scalar_mul(ot[:, 0:2 * w], tq_dst[:, 0:2 * w], out_scale)
            nc.scalar.dma_start(
                out=out_d[cb * 128:(cb + 1) * 128, i0:i0 + ni], in_=ot[:, 0:2 * w])

        def conv2_group(cb, chunks):
            # one group: chunks = list of (img_start, n_imgs); 4 m-banks per
            # chunk; stationary (ky, m, b, cb) reused across the chunks
            ps = {(m, i): psum.tile([128, 512], dt.float32, tag="ps", name="ps")
                  for m in range(4) for i in range(len(chunks))}
            for ky in range(3):
                for m in range(4):
                    for b in range(4):
                        w_ap = w2_t[b][:, ky, m, cb, :]
                        for i, (i0, ni) in enumerate(chunks):
                            rhs = v_t[:, m, b, i0:i0 + ni, ky:ky + 14, 0:7]
                            mm(ps[(m, i)][:, 0:ni * 98], w_ap, rhs,
                               start=(ky == 0 and b == 0),
                               stop=(ky == 2 and b == 3),
                               reuse=i > 0)
            for i, (i0, ni) in enumerate(chunks):
                conv2_epilogue(cb, i0, ni, [ps[(m, i)] for m in range(4)])

        for cb in range(4):
            if cb < 3:
                conv2_group(cb, [(0, 4), (4, 4)])
            else:
                # last cb: staggered, shrinking chunks so the final exposed
                # epilogue is a single image
                conv2_group(cb, [(0, 4)])
                conv2_group(cb, [(4, 2)])
                conv2_group(cb, [(6, 1)])
                conv2_group(cb, [(7, 1)])

    _dedupe_ldweights(nc)
    nc.compile()
    return nc


def _dedupe_ldweights(nc):
    """Drop LDWEIGHTS whose stationary operand is identical to the previous
    one on the PE stream (only MATMULs in between): the PE array keeps its
    loaded weights, so consecutive same-weight matmuls need a single load."""
    def sig_of(inst):
        a0 = inst.ins[0]
        try:
            return (a0.memref, a0.offset, str(a0.ap), str(a0.dtype))
        except Exception:
            return None

    removed = 0
    for blk in nc.main_func.blocks:
        last = None
        keep = []
        for inst in blk.instructions:
            tn = type(inst).__name__
            if inst.engine == mybir.EngineType.PE:
                if tn == "InstLdweights":
                    sig = sig_of(inst)
                    si = inst.sync_info
                    clean = si is None or (not si.on_wait and not si.on_update)
                    if sig is not None and sig == last and clean:
                        removed += 1
                        continue
                    last = sig
                elif tn != "InstMatmult":
                    last = None
            keep.append(inst)
        blk.instructions[:] = keep
    return removed


def prepare(x, w1, b1, w2, b2, in_scale, act1_scale, act2_scale):
    """Host-side prep: quantize weights, build per-core input maps + immediates."""
    x = np.asarray(x, np.float32)
    w1 = np.asarray(w1, np.float32)
    b1 = np.asarray(b1, np.float32)
    w2 = np.asarray(w2, np.float32)
    b2 = np.asarray(b2, np.float32)
    s_in = np.float32(np.asarray(in_scale).reshape(-1)[0])
    s_a1 = np.float32(np.asarray(act1_scale).reshape(-1)[0])
    s_a2 = np.float32(np.asarray(act2_scale).reshape(-1)[0])

    w1_int, s_w1 = _quant_weights(w1)
    w2_int, s_w2 = _quant_weights(w2)
    bq1 = np.clip(np.round(b1 / (s_in * s_w1)), -2.0 ** 31, 2.0 ** 31 - 1).astype(np.float32) * (s_in * s_w1)
    bq2 = np.clip(np.round(b2 / (s_a1 * s_w2)), -2.0 ** 31, 2.0 ** 31 - 1).astype(np.float32) * (s_a1 * s_w2)

    scale1 = float(np.float32(s_w1 / s_a1))
    scale2 = float(np.float32(s_a1 * s_w2 / s_a2))
    out_scale = float(s_a2)
    bias1 = np.ascontiguousarray((bq1 / s_a1).astype(np.float32).reshape(4, 128).T)  # (128, 4)
    bias2 = np.ascontiguousarray((bq2 / s_a2).astype(np.float32).reshape(4, 128).T)

    xp = _phase_planes(x)                                  # (64, 256, 2, 2, 15, 16)
    B, C = xp.shape[:2]
    pl = xp.reshape(B, C, 4, 15, 16)                       # plane = sr*2+sc
    pl_s = np.zeros_like(pl)
    pl_s[..., 0:15] = pl[..., 1:16]                        # shifted 1 col left
    # variant order matches _PVAR / DMA arrival order
    var = np.stack([pl[:, :, 3], pl[:, :, 1], pl_s[:, :, 2],
                    pl_s[:, :, 3], pl_s[:, :, 0], pl_s[:, :, 1]], axis=2)
    var16 = var.astype(np.float16)                         # (B, C, 6, 15, 16)

    w1_l = _w_lhsT(w1_int, 2).astype(np.int8)[:, :, TAP_ORDER]  # tap axis in demand order
    w2_l = _w2_wino(w2_int)                                # fp16 Winograd U

    in_maps = []
    for c in range(N_CORES):
        sl = slice(c * B_PER, (c + 1) * B_PER)
        m = {}
        for name, arr in (("x16", var16[sl]),):
            # (8, 256, 6, 15, 16) -> (ci_p 128, ci_blk 2, v 6, n 8, 15, 16)
            a = arr.transpose(1, 2, 0, 3, 4).reshape(2, 128, 6, B_PER, 15, 16)
            m[name] = np.ascontiguousarray(a.transpose(1, 0, 2, 3, 4, 5))
        m["w1"] = w1_l
        m["w2"] = w2_l
        m["b1"] = bias1
        m["b2"] = bias2
        in_maps.append(m)
    return (scale1, scale2, out_scale), in_maps


def gather_out(results):
    """Per-core (512, 8, 2par, 14, 7t) outputs -> full (64, 512, 14, 14); col = 2t+par."""
    out = np.empty((N_CORES * B_PER, 512, 14, 14), np.float32)
    for c, r in enumerate(results):
        o = np.asarray(r["out"])                           # (co, img, par, row, t)
        a = o.transpose(1, 0, 3, 4, 2).reshape(B_PER, 512, 14, 14)
        out[c * B_PER:(c + 1) * B_PER] = a
    return out


_cache = {}


def kernel(x, w1, b1, w2, b2, in_scale, act1_scale, act2_scale):
    imms, in_maps = prepare(x, w1, b1, w2, b2, in_scale, act1_scale, act2_scale)
    if imms not in _cache:
        _cache[imms] = build_program(*imms)
    nc = _cache[imms]
    res = run_bass_kernel_spmd(nc, in_maps, list(range(N_CORES)))
    return gather_out(res.results)



# revision 37
# speedup vs baseline: 1.0176x; 1.0176x over previous
"""Trainium2 Bass kernel for the quantized BasicBlock (conv3x3/s2 + fakequant + conv3x3/s1 + fakequant).

Sharding: data-parallel over batch across 8 cores (8 images each), weights replicated.

Device math (per core, B=8):
  conv1: implicit GEMM, 9 taps x 2 ci-blocks, single-pass fp16 (x rounded to fp16,
         ~11-bit mantissa; the act1 integer rounding absorbs the error well within
         the 2e-2 gate), integer-valued fp16 weights, fp32 PSUM accum.
  act1:  v = P1*(s_w1/s_a1) + bq1/s_a1; y = clip(rne(v), -128, 127) via the fp32
         magic-number trick on the DVE; y stored as integer-valued bf16 into a
         zero-padded [16x16] layout for conv2.
  conv2: 1-D Winograd F(2,3) along the column axis: V = B^T d (exact ints in
         bf16, 4 DVE ops per act chunk), U = G w (exact half-ints in fp16, host),
         GEMM over (ky, m) in fp32 PSUM (exact), inverse A^T on the DVE (exact
         ints) -> 1.5x fewer PE rows than direct.
  act2:  v2 = P2*(s_a1*s_w2/s_a2) + bq2/s_a2; out = clip(rne(v2), -128, 127) * s_a2.

Stride-2 conv1 is handled by a host-side phase split: x is scattered into 2x2 parity
planes zero-padded to 15x16 rows (32B-aligned rows); 6 plane variants (normal +
one-column-shifted) make every tap window start 4-byte aligned — misaligned rhs
windows cost ~15% per matmul on TRN2.

Input DMAs are chunked in first-use order on the Sync HWDGE queue while ~58 junk
warm-up matmuls cover the load latency and ramp the PE clock; outputs stream on
the Scalar queue.
"""
import os
import sys
from contextlib import ExitStack

import numpy as np
import ml_dtypes

for _p in ("/opt/trn_rl_repo",):
    if _p not in sys.path and os.path.isdir(_p):
        sys.path.insert(0, _p)

import concourse.bacc as bacc
import concourse.tile as tile
import concourse.mybir as mybir
from concourse.bass_utils import run_bass_kernel_spmd

BF16 = ml_dtypes.bfloat16
N_CORES = 8
B_PER = 8           # images per core
MAGIC = float(np.float32(1.5 * 2 ** 23))   # fp32 RNE rounding magic
Alu = mybir.AluOpType
dt = mybir.dt

# tap index k in {0,1,2} -> (parity s, window start offset) for the phase planes
_TAP = {0: (1, 0), 1: (0, 1), 2: (1, 1)}

# cb0 tap order, chosen so plane-variant demand follows DMA arrival order
TAP_ORDER = [0, 6, 3, 1, 7, 2, 8, 4, 5]
# (plane, col_offset) -> x_d variant index; 's' variants are pre-shifted one
# column left on the host so every window starts at column 0 (4B-aligned)
_PVAR = {(3, 0): 0, (1, 0): 1, (2, 1): 2, (3, 1): 3, (0, 1): 4, (1, 1): 5}
# w1's tap axis is permuted into TAP_ORDER on the host, so device-side w1
# indexing uses the order position
_TAP_POS = {t9: o for o, t9 in enumerate(TAP_ORDER)}


def _phase_planes(x):
    """(B, C, 28, 28) f32 -> (B, C, 2, 2, 15, 16): plane[sr][sc][q+1][p+1] = x[2q+sr][2p+sc].

    Rows are padded to 16 so SBUF row stride is 32 B (aligned); col 15 is
    never read by any tap window."""
    B, C = x.shape[:2]
    out = np.zeros((B, C, 2, 2, 15, 16), np.float32)
    for sr in (0, 1):
        for sc in (0, 1):
            out[:, :, sr, sc, 1:15, 1:15] = x[:, :, sr::2, sc::2]
    return out


def _quant_weights(w):
    """Per-tensor int8 narrow-range fake quant; returns (int-valued f32 weights, scale)."""
    s = np.float32(np.max(np.abs(w))) / np.float32(127.0)
    wq = np.clip(np.round(w / s), -127, 127).astype(np.float32)
    return wq, s


def _w2_wino(w_int):
    """(512co, 512ci, 3, 3) ints -> (ci_blk 4, 128, ky 3, m 4, cb 4, co 128) fp16.

    1-D Winograd F(2,3) weight transform along kx: U0 = g0, U1 = (g0+g1+g2)/2,
    U2 = (g0-g1+g2)/2, U3 = g2 — half-integers <= 190.5, exact in fp16."""
    g0 = w_int[..., 0]
    g1 = w_int[..., 1]
    g2 = w_int[..., 2]                                    # (co, ci, ky)
    U = np.stack([g0, (g0 + g1 + g2) * 0.5, (g0 - g1 + g2) * 0.5, g2], axis=0)
    t = U.transpose(2, 3, 0, 1)                           # (ci, ky, m, co)
    t = t.reshape(4, 128, 3, 4, 4, 128)                   # (ci_blk, ci, ky, m, cb, co)
    return np.ascontiguousarray(t).astype(np.float16)


def _w_lhsT(w_int, n_ci_blk):
    """(Cout=512, Cin, 3, 3) int-valued -> (ci_blk, 128, 9, 4, 128) bf16 stationary layout."""
    t = w_int.transpose(2, 3, 1, 0)                      # (3, 3, Cin, 512)
    t = t.reshape(9, n_ci_blk, 128, 4, 128)              # (tap, ci_blk, ci_p, co_blk, co)
    return np.ascontiguousarray(t.transpose(1, 2, 0, 3, 4)).astype(BF16)


_skip_ldw = [False]
_orig_InstMatmult = mybir.InstMatmult


def _patched_InstMatmult(*a, **kw):
    if _skip_ldw[0]:
        kw.setdefault("ldweights", False)
    return _orig_InstMatmult(*a, **kw)


def build_program(scale1, scale2, out_scale):
    """Build the (per-core SPMD) Bass program with the given fp32 immediates."""
    nc = bacc.Bacc("TRN2", target_bir_lowering=False, debug=False,
                   num_devices=N_CORES)

    mybir.InstMatmult = _patched_InstMatmult
    try:
        return _build_body(nc, scale1, scale2, out_scale)
    finally:
        mybir.InstMatmult = _orig_InstMatmult


def _build_body(nc, scale1, scale2, out_scale):
    NT = 4

    # 6 plane variants (normal / col-shifted) so every conv1 tap window starts
    # 4-byte aligned; order = DMA arrival order = cb0 tap demand order.
    x_d = nc.dram_tensor("x16", (128, 2, 6, B_PER, 15, 16), dt.float16, kind="ExternalInput")
    w1_d = nc.dram_tensor("w1", (2, 128, 9, 4, 128), dt.int8, kind="ExternalInput")
    w2_d = nc.dram_tensor("w2", (4, 128, 3, 4, 4, 128), dt.float16, kind="ExternalInput")
    b1_d = nc.dram_tensor("b1", (128, 4), dt.float32, kind="ExternalInput")
    b2_d = nc.dram_tensor("b2", (128, 4), dt.float32, kind="ExternalInput")
    out_d = nc.dram_tensor("out", (512, B_PER, 2, 14, 7), dt.float32, kind="ExternalOutput")

    def mm(out_ap, w_ap, rhs, start, stop, reuse):
        # reuse=True -> PE keeps the already-loaded stationary weights
        _skip_ldw[0] = reuse
        try:
            nc.tensor.matmul(out_ap, w_ap, rhs, start=start, stop=stop)
        finally:
            _skip_ldw[0] = False

    with tile.TileContext(nc) as tc, ExitStack() as ctx:
        const = ctx.enter_context(tc.tile_pool(name="const", bufs=1))
        psum = ctx.enter_context(tc.tile_pool(name="psum", bufs=8, space="PSUM"))
        tmp = ctx.enter_context(tc.tile_pool(name="tmp", bufs=2))
        outp = ctx.enter_context(tc.tile_pool(name="outp", bufs=2))

        # --- SBUF allocations: one tile per DMA chunk for fine-grained deps ---
        # x plane variants: [v][b] -> [128, n, 15, 16]
        x_t = [const.tile([128, 2, B_PER, 15, 16], dt.float16, tag=f"xh{v}", name=f"xh{v}")
               for v in range(6)]
        # w1: [b] -> [128, tap, co_blk, co]; weights arrive int8, DVE converts
        # them to fp16 (values are integers in [-127,127], exact either way)
        w1_t = [const.tile([128, 9, 4, 128], dt.float16, tag=f"w1{b}", name=f"w1t{b}") for b in range(2)]
        w2_t = [const.tile([128, 3, 4, 4, 128], dt.float16, tag=f"w2{b}", name=f"w2t{b}") for b in range(4)]
        w1i_t = [const.tile([128, 9, 4, 128], dt.int8, tag=f"w1i{b}", name=f"w1i{b}") for b in range(2)]
        b1_t = const.tile([128, 4], dt.float32, tag="b1")
        b2_t = const.tile([128, 4], dt.float32, tag="b2")
        act_t = const.tile([128, 4, B_PER, 16, 16], dt.bfloat16, tag="act")  # padded act1
        # Winograd-domain act: V[m][cb][img][row][tile], tile dim padded to 8
        # so every conv2 rhs window starts 16B-aligned
        v_t = const.tile([128, 4, 4, B_PER, 16, 8], dt.bfloat16, tag="vt")
        wz = const.tile([128, 256], dt.bfloat16, tag="wz")

        # PE warm-up source zeros; act pad memsets go on the DVE *after* the
        # w1 casts (emitted below) — gpsimd memsets are slow and their SBUF
        # traffic stalls the startup casts, while the DVE is idle from cast
        # end (~20us) until the first act1 epilogue (~29us).
        nc.vector.memset(wz[:], 0.0)

        # --- input loads in first-use order on the Sync HWDGE queue (the two
        # HWDGE queues share HBM bandwidth, so splitting input across both
        # gains nothing; outputs use the Scalar queue) ---
        def load(dst, src):
            nc.sync.dma_start(out=dst, in_=src)

        def load_plane(v):
            load(x_t[v][:], x_d[:, :, v])

        # w1 first (small); tap axis is TAP_ORDER-permuted on host and the
        # int8->fp16 casts are chunked per (tap, ci_blk) in demand order
        for b in range(2):
            load(w1i_t[b][:], w1_d[b])
        for o in range(9):
            for b in range(2):
                nc.vector.tensor_copy(w1_t[b][:, o], w1i_t[b][:, o])
        nc.vector.memset(act_t[:], 0.0)
        load_plane(0)
        load(b1_t[:], b1_d[:])
        for v in range(1, 6):
            load_plane(v)
        for b in range(4):
            load(w2_t[b][:], w2_d[b])
        load(b2_t[:], b2_d[:])

        def quant_chain(dst, src, sc, bias_ap, width=392):
            """dst = clip(rne(src*sc + bias), -128, 127) on the DVE (3 fused ops)."""
            tt = tmp.tile([128, width], dt.float32, tag=f"tt{width}", name="tt")
            nc.vector.tensor_scalar(tt[:], src, sc, bias_ap, op0=Alu.mult, op1=Alu.add)
            nc.vector.tensor_scalar(tt[:], tt[:], MAGIC, MAGIC + 127.0, op0=Alu.add, op1=Alu.min)
            nc.vector.tensor_scalar(dst, tt[:], MAGIC - 128.0, -MAGIC, op0=Alu.max, op1=Alu.add)
            return tt

        # PE warm-up: junk matmuls on the zeroed tile during the input-DMA wait
        # so the HAM clock gate is at full rate when the real stream starts.
        wps = psum.tile([128, 512], dt.float32, tag="ps", name="warmps")
        for i in range(58):
            nc.tensor.matmul(wps[:, 0:256], wz[:, 0:128], wz[:, 0:256],
                             start=True, stop=True)

        # --- conv1 + act1 ---
        # cb0 is tap-major: plane demand spread over the whole 144-MM group to
        # match DMA delivery. cb1-3 are nt-major: each psum bank finishes early
        # and its epilogue overlaps the remaining banks' matmuls.
        def conv1_group(cb, t9, b, ps_list, nts):
            # one stationary weight (t9, b, cb) serving len(nts) matmuls;
            # only the first self-loads the PE array
            ky, kx = divmod(t9, 3)
            sr, r0 = _TAP[ky]
            sc_, c0 = _TAP[kx]
            v = _PVAR[(sr * 2 + sc_, c0)]
            w_ap = w1_t[b][:, _TAP_POS[t9], cb, :]
            for i, nt in enumerate(nts):
                rhs = x_t[v][:, b, 2 * nt:2 * nt + 2, r0:r0 + 14, 0:14]
                mm(ps_list[i][:, 0:392], w_ap, rhs,
                   start=(t9 == TAP_ORDER[0] and b == 0),
                   stop=(t9 == TAP_ORDER[-1] and b == 1),
                   reuse=i > 0)

        def act1_chunk(cb, nt, ps):
            quant_chain(act_t[:, cb, 2 * nt:2 * nt + 2, 1:15, 1:15],
                        ps[:, 0:392], scale1, b1_t[:, cb:cb + 1])
            # 1-D Winograd data transform V = B^T d over the column axis:
            # V0 = d0-d2, V1 = d1+d2, V2 = d2-d1, V3 = d1-d3 (per 4-col tile,
            # stride 2; all 16 rows incl. pads; exact small ints in bf16)
            a = act_t[:, cb, 2 * nt:2 * nt + 2, :, :]
            ev0 = a[:, :, :, 0:13:2]
            od1 = a[:, :, :, 1:14:2]
            ev2 = a[:, :, :, 2:15:2]
            od3 = a[:, :, :, 3:16:2]
            dst = lambda m: v_t[:, m, cb, 2 * nt:2 * nt + 2, :, 0:7]
            nc.vector.tensor_tensor(dst(0), ev0, ev2, op=Alu.subtract)
            nc.vector.tensor_tensor(dst(1), od1, ev2, op=Alu.add)
            nc.vector.tensor_tensor(dst(2), ev2, od1, op=Alu.subtract)
            nc.vector.tensor_tensor(dst(3), od1, od3, op=Alu.subtract)

        for cb in range(4):
            if cb == 0:
                # tap-major: plane demand spread over the whole group to match
                # the DMA delivery ramp; 8 matmuls per weight load
                ps_n = [psum.tile([128, 512], dt.float32, tag="ps", name="ps")
                        for _ in range(NT)]
                for t9 in TAP_ORDER:
                    for b in range(2):
                        conv1_group(cb, t9, b, ps_n, range(NT))
                for nt in range(NT):
                    act1_chunk(cb, nt, ps_n[nt])
            else:
                # nt-pair-major: each bank pair finishes at half-time so its
                # epilogue overlaps the rest; the last cb runs single-nt
                # groups so its final epilogue chain is short (conv2's first
                # taps wait on it)
                halves = ([[0, 1], [2, 3]] if cb < 3 else [[0], [1], [2], [3]])
                for nts in halves:
                    ps_p = [psum.tile([128, 512], dt.float32, tag="ps", name="ps")
                            for _ in nts]
                    for t9 in TAP_ORDER:
                        for b in range(2):
                            conv1_group(cb, t9, b, ps_p, nts)
                    for i, nt in enumerate(nts):
                        act1_chunk(cb, nt, ps_p[i])

        # --- conv2 (1-D Winograd) + act2 ---
        # For each (out-cb, image-pair chunk): 8 PSUM banks hold the 4 m-
        # positions x 2 chunks; GEMM accumulates over (ky, ci-blk). The
        # epilogue applies the inverse transform A^T (o0 = m0+m1+m2,
        # o1 = m1-m2-m3, both exact ints in fp32), then the act2 quant chain.
        def conv2_epilogue(cb, i0, ni, psm):
            # psm: list of 4 PSUM tiles [128, ni*98] (m = 0..3) for images
            # i0..i0+ni (ni <= 4). Only one PSUM operand is allowed per DVE
            # op, so m1 is staged to SBUF first.
            w = ni * 98
            tq = tmp.tile([128, 4, 2, 14, 7], dt.float32, tag="tq", name="tq")
            s1 = tmp.tile([128, 392], dt.float32, tag="s1", name="s1")
            ti = tmp.tile([128, 392], dt.float32, tag="ti", name="ti")
            t2 = tmp.tile([128, 392], dt.float32, tag="t2", name="t2")
            nc.vector.tensor_copy(s1[:, 0:w], psm[1][:, 0:w])
            nc.vector.tensor_tensor(ti[:, 0:w], psm[0][:, 0:w], s1[:, 0:w], op=Alu.add)
            nc.vector.tensor_tensor(tq[:, 0:ni, 0], ti[:, 0:w], psm[2][:, 0:w], op=Alu.add)
            nc.vector.tensor_tensor(t2[:, 0:w], s1[:, 0:w], psm[2][:, 0:w], op=Alu.subtract)
            nc.vector.tensor_tensor(tq[:, 0:ni, 1], t2[:, 0:w], psm[3][:, 0:w], op=Alu.subtract)
            ot = outp.tile([128, 784], dt.float32, tag="ot", name="ot")
            tq_dst = tmp.tile([128, 784], dt.float32, tag="tq2", name="tq2")
            quant_chain(tq_dst[:, 0:2 * w], tq[:, 0:ni], scale2, b2_t[:, cb:cb + 1],
                        width=2 * w)
            nc.vector.tensor_scalar_mul(ot[:, 0:2 * w], tq_dst[:, 0:2 * w], out_scale)
            nc.scalar.dma_start(
                out=out_d[cb * 128:(cb + 1) * 128, i0:i0 + ni], in_=ot[:, 0:2 * w])

        def conv2_group(cb, chunks):
            # one group: chunks = list of (img_start, n_imgs); 4 m-banks per
            # chunk; stationary (ky, m, b, cb) reused across the chunks
            ps = {(m, i): psum.tile([128, 512], dt.float32, tag="ps", name="ps")
                  for m in range(4) for i in range(len(chunks))}
            for ky in range(3):
                for m in range(4):
                    for b in range(4):
                        w_ap = w2_t[b][:, ky, m, cb, :]
                        for i, (i0, ni) in enumerate(chunks):
                            rhs = v_t[:, m, b, i0:i0 + ni, ky:ky + 14, 0:7]
                            mm(ps[(m, i)][:, 0:ni * 98], w_ap, rhs,
                               start=(ky == 0 and b == 0),
                               stop=(ky == 2 and b == 3),
                               reuse=i > 0)
            for i, (i0, ni) in enumerate(chunks):
                conv2_epilogue(cb, i0, ni, [ps[(m, i)] for m in range(4)])

        for cb in range(4):
            if cb < 3:
                conv2_group(cb, [(0, 4), (4, 4)])
            else:
                # last cb: staggered, shrinking chunks so the final exposed
                # epilogue is a single image
                conv2_group(cb, [(0, 4)])
                conv2_group(cb, [(4, 2)])
                conv2_group(cb, [(6, 1)])
                conv2_group(cb, [(7, 1)])

    _dedupe_ldweights(nc)
    nc.compile()
    return nc


def _dedupe_ldweights(nc):
    """Drop LDWEIGHTS whose stationary operand is identical to the previous
    one on the PE stream (only MATMULs in between): the PE array keeps its
    loaded weights, so consecutive same-weight matmuls need a single load."""
    def sig_of(inst):
        a0 = inst.ins[0]
        try:
            return (a0.memref, a0.offset, str(a0.ap), str(a0.dtype))
        except Exception:
            return None

    removed = 0
    for blk in nc.main_func.blocks:
        last = None
        keep = []
        for inst in blk.instructions:
            tn = type(inst).__name__
            if inst.engine == mybir.EngineType.PE:
                if tn == "InstLdweights":
                    sig = sig_of(inst)
                    si = inst.sync_info
                    clean = si is None or (not si.on_wait and not si.on_update)
                    if sig is not None and sig == last and clean:
                        removed += 1
                        continue
                    last = sig
                elif tn != "InstMatmult":
                    last = None
            keep.append(inst)
        blk.instructions[:] = keep
    return removed


def prepare(x, w1, b1, w2, b2, in_scale, act1_scale, act2_scale):
    """Host-side prep: quantize weights, build per-core input maps + immediates."""
    x = np.asarray(x, np.float32)
    w1 = np.asarray(w1, np.float32)
    b1 = np.asarray(b1, np.float32)
    w2 = np.asarray(w2, np.float32)
    b2 = np.asarray(b2, np.float32)
    s_in = np.float32(np.asarray(in_scale).reshape(-1)[0])
    s_a1 = np.float32(np.asarray(act1_scale).reshape(-1)[0])
    s_a2 = np.float32(np.asarray(act2_scale).reshape(-1)[0])

    w1_int, s_w1 = _quant_weights(w1)
    w2_int, s_w2 = _quant_weights(w2)
    bq1 = np.clip(np.round(b1 / (s_in * s_w1)), -2.0 ** 31, 2.0 ** 31 - 1).astype(np.float32) * (s_in * s_w1)
    bq2 = np.clip(np.round(b2 / (s_a1 * s_w2)), -2.0 ** 31, 2.0 ** 31 - 1).astype(np.float32) * (s_a1 * s_w2)

    scale1 = float(np.float32(s_w1 / s_a1))
    scale2 = float(np.float32(s_a1 * s_w2 / s_a2))
    out_scale = float(s_a2)
    bias1 = np.ascontiguousarray((bq1 / s_a1).astype(np.float32).reshape(4, 128).T)  # (128, 4)
    bias2 = np.ascontiguousarray((bq2 / s_a2).astype(np.float32).reshape(4, 128).T)

    xp = _phase_planes(x)                                  # (64, 256, 2, 2, 15, 16)
    B, C = xp.shape[:2]
    pl = xp.reshape(B, C, 4, 15, 16)                       # plane = sr*2+sc
    pl_s = np.zeros_like(pl)
    pl_s[..., 0:15] = pl[..., 1:16]                        # shifted 1 col left
    # variant order matches _PVAR / DMA arrival order
    var = np.stack([pl[:, :, 3], pl[:, :, 1], pl_s[:, :, 2],
                    pl_s[:, :, 3], pl_s[:, :, 0], pl_s[:, :, 1]], axis=2)
    var16 = var.astype(np.float16)                         # (B, C, 6, 15, 16)

    w1_l = _w_lhsT(w1_int, 2).astype(np.int8)[:, :, TAP_ORDER]  # tap axis in demand order
    w2_l = _w2_wino(w2_int)                                # fp16 Winograd U

    in_maps = []
    for c in range(N_CORES):
        sl = slice(c * B_PER, (c + 1) * B_PER)
        m = {}
        for name, arr in (("x16", var16[sl]),):
            # (8, 256, 6, 15, 16) -> (ci_p 128, ci_blk 2, v 6, n 8, 15, 16)
            a = arr.transpose(1, 2, 0, 3, 4).reshape(2, 128, 6, B_PER, 15, 16)
            m[name] = np.ascontiguousarray(a.transpose(1, 0, 2, 3, 4, 5))
        m["w1"] = w1_l
        m["w2"] = w2_l
        m["b1"] = bias1
        m["b2"] = bias2
        in_maps.append(m)
    return (scale1, scale2, out_scale), in_maps


def gather_out(results):
    """Per-core (512, 8, 2par, 14, 7t) outputs -> full (64, 512, 14, 14); col = 2t+par."""
    out = np.empty((N_CORES * B_PER, 512, 14, 14), np.float32)
    for c, r in enumerate(results):
        o = np.asarray(r["out"])                           # (co, img, par, row, t)
        a = o.transpose(1, 0, 3, 4, 2).reshape(B_PER, 512, 14, 14)
        out[c * B_PER:(c + 1) * B_PER] = a
    return out


_cache = {}


def kernel(x, w1, b1, w2, b2, in_scale, act1_scale, act2_scale):
    imms, in_maps = prepare(x, w1, b1, w2, b2, in_scale, act1_scale, act2_scale)
    if imms not in _cache:
        _cache[imms] = build_program(*imms)
    nc = _cache[imms]
    res = run_bass_kernel_spmd(nc, in_maps, list(range(N_CORES)))
    return gather_out(res.results)



# revision 38
# speedup vs baseline: 1.0219x; 1.0042x over previous
"""Trainium2 Bass kernel for the quantized BasicBlock (conv3x3/s2 + fakequant + conv3x3/s1 + fakequant).

Sharding: data-parallel over batch across 8 cores (8 images each), weights replicated.

Device math (per core, B=8):
  conv1: implicit GEMM, 9 taps x 2 ci-blocks, single-pass fp16 (x rounded to fp16,
         ~11-bit mantissa; the act1 integer rounding absorbs the error well within
         the 2e-2 gate), integer-valued fp16 weights, fp32 PSUM accum.
  act1:  v = P1*(s_w1/s_a1) + bq1/s_a1; y = clip(rne(v), -128, 127) via the fp32
         magic-number trick on the DVE; y stored as integer-valued bf16 into a
         zero-padded [16x16] layout for conv2.
  conv2: 1-D Winograd F(2,3) along the column axis: V = B^T d (exact ints in
         bf16, 4 DVE ops per act chunk), U = G w (exact half-ints in fp16, host),
         GEMM over (ky, m) in fp32 PSUM (exact), inverse A^T on the DVE (exact
         ints) -> 1.5x fewer PE rows than direct.
  act2:  v2 = P2*(s_a1*s_w2/s_a2) + bq2/s_a2; out = clip(rne(v2), -128, 127) * s_a2.

Stride-2 conv1 is handled by a host-side phase split: x is scattered into 2x2 parity
planes zero-padded to 15x16 rows (32B-aligned rows); 6 plane variants (normal +
one-column-shifted) make every tap window start 4-byte aligned — misaligned rhs
windows cost ~15% per matmul on TRN2.

Input DMAs are chunked in first-use order on the Sync HWDGE queue while ~58 junk
warm-up matmuls cover the load latency and ramp the PE clock; outputs stream on
the Scalar queue.
"""
import os
import sys
from contextlib import ExitStack

import numpy as np
import ml_dtypes

for _p in ("/opt/trn_rl_repo",):
    if _p not in sys.path and os.path.isdir(_p):
        sys.path.insert(0, _p)

import concourse.bacc as bacc
import concourse.tile as tile
import concourse.mybir as mybir
from concourse.bass_utils import run_bass_kernel_spmd

BF16 = ml_dtypes.bfloat16
N_CORES = 8
B_PER = 8           # images per core
MAGIC = float(np.float32(1.5 * 2 ** 23))   # fp32 RNE rounding magic
Alu = mybir.AluOpType
dt = mybir.dt

# tap index k in {0,1,2} -> (parity s, window start offset) for the phase planes
_TAP = {0: (1, 0), 1: (0, 1), 2: (1, 1)}

# cb0 tap order, chosen so plane-variant demand follows DMA arrival order
TAP_ORDER = [0, 6, 3, 1, 7, 2, 8, 4, 5]
# (plane, col_offset) -> x_d variant index; 's' variants are pre-shifted one
# column left on the host so every window starts at column 0 (4B-aligned)
_PVAR = {(3, 0): 0, (1, 0): 1, (2, 1): 2, (3, 1): 3, (0, 1): 4, (1, 1): 5}
# w1's tap axis is permuted into TAP_ORDER on the host, so device-side w1
# indexing uses the order position
_TAP_POS = {t9: o for o, t9 in enumerate(TAP_ORDER)}


def _phase_planes(x):
    """(B, C, 28, 28) f32 -> (B, C, 2, 2, 15, 16): plane[sr][sc][q+1][p+1] = x[2q+sr][2p+sc].

    Rows are padded to 16 so SBUF row stride is 32 B (aligned); col 15 is
    never read by any tap window."""
    B, C = x.shape[:2]
    out = np.zeros((B, C, 2, 2, 15, 16), np.float32)
    for sr in (0, 1):
        for sc in (0, 1):
            out[:, :, sr, sc, 1:15, 1:15] = x[:, :, sr::2, sc::2]
    return out


def _quant_weights(w):
    """Per-tensor int8 narrow-range fake quant; returns (int-valued f32 weights, scale)."""
    s = np.float32(np.max(np.abs(w))) / np.float32(127.0)
    wq = np.clip(np.round(w / s), -127, 127).astype(np.float32)
    return wq, s


def _w2_wino(w_int):
    """(512co, 512ci, 3, 3) ints -> (ci_blk 4, 128, ky 3, m 4, cb 4, co 128) fp16.

    1-D Winograd F(2,3) weight transform along kx: U0 = g0, U1 = (g0+g1+g2)/2,
    U2 = (g0-g1+g2)/2, U3 = g2 — half-integers <= 190.5, exact in fp16."""
    g0 = w_int[..., 0]
    g1 = w_int[..., 1]
    g2 = w_int[..., 2]                                    # (co, ci, ky)
    U = np.stack([g0, (g0 + g1 + g2) * 0.5, (g0 - g1 + g2) * 0.5, g2], axis=0)
    t = U.transpose(2, 3, 0, 1)                           # (ci, ky, m, co)
    t = t.reshape(4, 128, 3, 4, 4, 128)                   # (ci_blk, ci, ky, m, cb, co)
    return np.ascontiguousarray(t).astype(np.float16)


def _w_lhsT(w_int, n_ci_blk):
    """(Cout=512, Cin, 3, 3) int-valued -> (ci_blk, 128, 9, 4, 128) bf16 stationary layout."""
    t = w_int.transpose(2, 3, 1, 0)                      # (3, 3, Cin, 512)
    t = t.reshape(9, n_ci_blk, 128, 4, 128)              # (tap, ci_blk, ci_p, co_blk, co)
    return np.ascontiguousarray(t.transpose(1, 2, 0, 3, 4)).astype(BF16)


_skip_ldw = [False]
_orig_InstMatmult = mybir.InstMatmult


def _patched_InstMatmult(*a, **kw):
    if _skip_ldw[0]:
        kw.setdefault("ldweights", False)
    return _orig_InstMatmult(*a, **kw)


def build_program(scale1, scale2, out_scale):
    """Build the (per-core SPMD) Bass program with the given fp32 immediates."""
    nc = bacc.Bacc("TRN2", target_bir_lowering=False, debug=False,
                   num_devices=N_CORES)

    mybir.InstMatmult = _patched_InstMatmult
    try:
        return _build_body(nc, scale1, scale2, out_scale)
    finally:
        mybir.InstMatmult = _orig_InstMatmult


def _build_body(nc, scale1, scale2, out_scale):
    NT = 4

    # 6 plane variants (normal / col-shifted) so every conv1 tap window starts
    # 4-byte aligned; order = DMA arrival order = cb0 tap demand order.
    x_d = nc.dram_tensor("x16", (128, 2, 6, B_PER, 15, 16), dt.float16, kind="ExternalInput")
    w1_d = nc.dram_tensor("w1", (2, 128, 9, 4, 128), dt.int8, kind="ExternalInput")
    w2_d = nc.dram_tensor("w2", (4, 128, 3, 4, 4, 128), dt.float16, kind="ExternalInput")
    b1_d = nc.dram_tensor("b1", (128, 4), dt.float32, kind="ExternalInput")
    b2_d = nc.dram_tensor("b2", (128, 4), dt.float32, kind="ExternalInput")
    out_d = nc.dram_tensor("out", (512, B_PER, 2, 14, 7), dt.float32, kind="ExternalOutput")

    def mm(out_ap, w_ap, rhs, start, stop, reuse):
        # reuse=True -> PE keeps the already-loaded stationary weights
        _skip_ldw[0] = reuse
        try:
            nc.tensor.matmul(out_ap, w_ap, rhs, start=start, stop=stop)
        finally:
            _skip_ldw[0] = False

    with tile.TileContext(nc) as tc, ExitStack() as ctx:
        const = ctx.enter_context(tc.tile_pool(name="const", bufs=1))
        psum = ctx.enter_context(tc.tile_pool(name="psum", bufs=8, space="PSUM"))
        tmp = ctx.enter_context(tc.tile_pool(name="tmp", bufs=2))
        outp = ctx.enter_context(tc.tile_pool(name="outp", bufs=2))

        # --- SBUF allocations: one tile per DMA chunk for fine-grained deps ---
        # x plane variants: [v][b] -> [128, n, 15, 16]
        x_t = [const.tile([128, 2, B_PER, 15, 16], dt.float16, tag=f"xh{v}", name=f"xh{v}")
               for v in range(6)]
        # w1: [b] -> [128, tap, co_blk, co]; weights arrive int8, DVE converts
        # them to fp16 (values are integers in [-127,127], exact either way)
        w1_t = [const.tile([128, 9, 4, 128], dt.float16, tag=f"w1{b}", name=f"w1t{b}") for b in range(2)]
        w2_t = [const.tile([128, 3, 4, 4, 128], dt.float16, tag=f"w2{b}", name=f"w2t{b}") for b in range(4)]
        w1i_t = [const.tile([128, 9, 4, 128], dt.int8, tag=f"w1i{b}", name=f"w1i{b}") for b in range(2)]
        b1_t = const.tile([128, 4], dt.float32, tag="b1")
        b2_t = const.tile([128, 4], dt.float32, tag="b2")
        act_t = const.tile([128, 4, B_PER, 16, 16], dt.bfloat16, tag="act")  # padded act1
        # Winograd-domain act: V[m][cb][img][row][tile], tile dim padded to 8
        # so every conv2 rhs window starts 16B-aligned
        v_t = const.tile([128, 4, 4, B_PER, 16, 8], dt.bfloat16, tag="vt")
        wz = const.tile([128, 256], dt.bfloat16, tag="wz")

        # PE warm-up source zeros; act pad memsets go on the DVE *after* the
        # w1 casts (emitted below) — gpsimd memsets are slow and their SBUF
        # traffic stalls the startup casts, while the DVE is idle from cast
        # end (~20us) until the first act1 epilogue (~29us).
        nc.vector.memset(wz[:], 0.0)

        # --- input loads in first-use order on the Sync HWDGE queue (the two
        # HWDGE queues share HBM bandwidth, so splitting input across both
        # gains nothing; outputs use the Scalar queue) ---
        def load(dst, src):
            nc.sync.dma_start(out=dst, in_=src)

        def load_plane(v):
            load(x_t[v][:], x_d[:, :, v])

        # w1 first (small); tap axis is TAP_ORDER-permuted on host and the
        # int8->fp16 casts are chunked per (tap, ci_blk) in demand order
        for b in range(2):
            load(w1i_t[b][:], w1_d[b])
        for o in range(9):
            for b in range(2):
                nc.vector.tensor_copy(w1_t[b][:, o], w1i_t[b][:, o])
        nc.vector.memset(act_t[:], 0.0)
        load_plane(0)
        load(b1_t[:], b1_d[:])
        for v in range(1, 6):
            load_plane(v)
        for b in range(4):
            load(w2_t[b][:], w2_d[b])
        load(b2_t[:], b2_d[:])

        def quant_chain(dst, src, sc, bias_ap, width=392):
            """dst = clip(rne(src*sc + bias), -128, 127) on the DVE (3 fused ops)."""
            tt = tmp.tile([128, width], dt.float32, tag=f"tt{width}", name="tt")
            nc.vector.tensor_scalar(tt[:], src, sc, bias_ap, op0=Alu.mult, op1=Alu.add)
            nc.vector.tensor_scalar(tt[:], tt[:], MAGIC, MAGIC + 127.0, op0=Alu.add, op1=Alu.min)
            nc.vector.tensor_scalar(dst, tt[:], MAGIC - 128.0, -MAGIC, op0=Alu.max, op1=Alu.add)
            return tt

        # PE warm-up: junk matmuls on the zeroed tile during the input-DMA wait
        # so the HAM clock gate is at full rate when the real stream starts.
        wps = psum.tile([128, 512], dt.float32, tag="ps", name="warmps")
        for i in range(58):
            nc.tensor.matmul(wps[:, 0:256], wz[:, 0:128], wz[:, 0:256],
                             start=True, stop=True)

        # --- conv1 + act1 ---
        # cb0 is tap-major: plane demand spread over the whole 144-MM group to
        # match DMA delivery. cb1-3 are nt-major: each psum bank finishes early
        # and its epilogue overlaps the remaining banks' matmuls.
        def conv1_group(cb, t9, b, ps_list, nts):
            # one stationary weight (t9, b, cb) serving len(nts) matmuls;
            # only the first self-loads the PE array
            ky, kx = divmod(t9, 3)
            sr, r0 = _TAP[ky]
            sc_, c0 = _TAP[kx]
            v = _PVAR[(sr * 2 + sc_, c0)]
            w_ap = w1_t[b][:, _TAP_POS[t9], cb, :]
            for i, nt in enumerate(nts):
                rhs = x_t[v][:, b, 2 * nt:2 * nt + 2, r0:r0 + 14, 0:14]
                mm(ps_list[i][:, 0:392], w_ap, rhs,
                   start=(t9 == TAP_ORDER[0] and b == 0),
                   stop=(t9 == TAP_ORDER[-1] and b == 1),
                   reuse=i > 0)

        def act1_chunk(cb, nt, ps):
            quant_chain(act_t[:, cb, 2 * nt:2 * nt + 2, 1:15, 1:15],
                        ps[:, 0:392], scale1, b1_t[:, cb:cb + 1])
            # 1-D Winograd data transform V = B^T d over the column axis:
            # V0 = d0-d2, V1 = d1+d2, V2 = d2-d1, V3 = d1-d3 (per 4-col tile,
            # stride 2; all 16 rows incl. pads; exact small ints in bf16)
            a = act_t[:, cb, 2 * nt:2 * nt + 2, :, :]
            ev0 = a[:, :, :, 0:13:2]
            od1 = a[:, :, :, 1:14:2]
            ev2 = a[:, :, :, 2:15:2]
            od3 = a[:, :, :, 3:16:2]
            dst = lambda m: v_t[:, m, cb, 2 * nt:2 * nt + 2, :, 0:7]
            nc.vector.tensor_tensor(dst(0), ev0, ev2, op=Alu.subtract)
            nc.vector.tensor_tensor(dst(1), od1, ev2, op=Alu.add)
            nc.vector.tensor_tensor(dst(2), ev2, od1, op=Alu.subtract)
            nc.vector.tensor_tensor(dst(3), od1, od3, op=Alu.subtract)

        for cb in range(4):
            if cb == 0:
                # tap-major: plane demand spread over the whole group to match
                # the DMA delivery ramp; 8 matmuls per weight load
                ps_n = [psum.tile([128, 512], dt.float32, tag="ps", name="ps")
                        for _ in range(NT)]
                for t9 in TAP_ORDER:
                    for b in range(2):
                        conv1_group(cb, t9, b, ps_n, range(NT))
                for nt in range(NT):
                    act1_chunk(cb, nt, ps_n[nt])
            else:
                # nt-pair-major: each bank pair finishes at half-time so its
                # epilogue overlaps the rest; the last cb runs single-nt
                # groups so its final epilogue chain is short (conv2's first
                # taps wait on it)
                halves = ([[0, 1], [2, 3]] if cb < 3 else [[0], [1], [2], [3]])
                for nts in halves:
                    ps_p = [psum.tile([128, 512], dt.float32, tag="ps", name="ps")
                            for _ in nts]
                    for t9 in TAP_ORDER:
                        for b in range(2):
                            conv1_group(cb, t9, b, ps_p, nts)
                    for i, nt in enumerate(nts):
                        act1_chunk(cb, nt, ps_p[i])

        # --- conv2 (1-D Winograd) + act2 ---
        # For each (out-cb, image-pair chunk): 8 PSUM banks hold the 4 m-
        # positions x 2 chunks; GEMM accumulates over (ky, ci-blk). The
        # epilogue applies the inverse transform A^T (o0 = m0+m1+m2,
        # o1 = m1-m2-m3, both exact ints in fp32), then the act2 quant chain.
        def conv2_epilogue(cb, i0, ni, psm):
            # psm: list of 4 PSUM tiles [128, ni*98] (m = 0..3) for images
            # i0..i0+ni (ni <= 4). Only one PSUM operand is allowed per DVE
            # op, so m1 is staged to SBUF first.
            w = ni * 98
            tq = tmp.tile([128, 4, 2, 14, 7], dt.float32, tag="tq", name="tq")
            s1 = tmp.tile([128, 392], dt.float32, tag="s1", name="s1")
            ti = tmp.tile([128, 392], dt.float32, tag="ti", name="ti")
            t2 = tmp.tile([128, 392], dt.float32, tag="t2", name="t2")
            nc.vector.tensor_copy(s1[:, 0:w], psm[1][:, 0:w])
            nc.vector.tensor_tensor(ti[:, 0:w], psm[0][:, 0:w], s1[:, 0:w], op=Alu.add)
            nc.vector.tensor_tensor(tq[:, 0:ni, 0], ti[:, 0:w], psm[2][:, 0:w], op=Alu.add)
            nc.vector.tensor_tensor(t2[:, 0:w], s1[:, 0:w], psm[2][:, 0:w], op=Alu.subtract)
            nc.vector.tensor_tensor(tq[:, 0:ni, 1], t2[:, 0:w], psm[3][:, 0:w], op=Alu.subtract)
            ot = outp.tile([128, 784], dt.float32, tag="ot", name="ot")
            tq_dst = tmp.tile([128, 784], dt.float32, tag="tq2", name="tq2")
            quant_chain(tq_dst[:, 0:2 * w], tq[:, 0:ni], scale2, b2_t[:, cb:cb + 1],
                        width=2 * w)
            nc.vector.tensor_scalar_mul(ot[:, 0:2 * w], tq_dst[:, 0:2 * w], out_scale)
            nc.scalar.dma_start(
                out=out_d[cb * 128:(cb + 1) * 128, i0:i0 + ni], in_=ot[:, 0:2 * w])

        def conv2_group(cb, chunks):
            # one group: chunks = list of (img_start, n_imgs); 4 m-banks per
            # chunk; stationary (ky, m, b, cb) reused across the chunks
            ps = {(m, i): psum.tile([128, 512], dt.float32, tag="ps", name="ps")
                  for m in range(4) for i in range(len(chunks))}
            # m outermost: ps[0..2] finish their accumulation 12-36 matmuls
            # before the group ends, so the epilogue's inverse-transform
            # prefix overlaps the tail of the matmul stream
            for m in range(4):
                for ky in range(3):
                    for b in range(4):
                        w_ap = w2_t[b][:, ky, m, cb, :]
                        for i, (i0, ni) in enumerate(chunks):
                            rhs = v_t[:, m, b, i0:i0 + ni, ky:ky + 14, 0:7]
                            mm(ps[(m, i)][:, 0:ni * 98], w_ap, rhs,
                               start=(ky == 0 and b == 0),
                               stop=(ky == 2 and b == 3),
                               reuse=i > 0)
            for i, (i0, ni) in enumerate(chunks):
                conv2_epilogue(cb, i0, ni, [ps[(m, i)] for m in range(4)])

        for cb in range(4):
            if cb < 3:
                conv2_group(cb, [(0, 4), (4, 4)])
            else:
                # last cb: staggered, shrinking chunks so the final exposed
                # epilogue is a single image
                conv2_group(cb, [(0, 4)])
                conv2_group(cb, [(4, 2)])
                conv2_group(cb, [(6, 1)])
                conv2_group(cb, [(7, 1)])

    _dedupe_ldweights(nc)
    nc.compile()
    return nc


def _dedupe_ldweights(nc):
    """Drop LDWEIGHTS whose stationary operand is identical to the previous
    one on the PE stream (only MATMULs in between): the PE array keeps its
    loaded weights, so consecutive same-weight matmuls need a single load."""
    def sig_of(inst):
        a0 = inst.ins[0]
        try:
            return (a0.memref, a0.offset, str(a0.ap), str(a0.dtype))
        except Exception:
            return None

    removed = 0
    for blk in nc.main_func.blocks:
        last = None
        keep = []
        for inst in blk.instructions:
            tn = type(inst).__name__
            if inst.engine == mybir.EngineType.PE:
                if tn == "InstLdweights":
                    sig = sig_of(inst)
                    si = inst.sync_info
                    clean = si is None or (not si.on_wait and not si.on_update)
                    if sig is not None and sig == last and clean:
                        removed += 1
                        continue
                    last = sig
                elif tn != "InstMatmult":
                    last = None
            keep.append(inst)
        blk.instructions[:] = keep
    return removed


def prepare(x, w1, b1, w2, b2, in_scale, act1_scale, act2_scale):
    """Host-side prep: quantize weights, build per-core input maps + immediates."""
    x = np.asarray(x, np.float32)
    w1 = np.asarray(w1, np.float32)
    b1 = np.asarray(b1, np.float32)
    w2 = np.asarray(w2, np.float32)
    b2 = np.asarray(b2, np.float32)
    s_in = np.float32(np.asarray(in_scale).reshape(-1)[0])
    s_a1 = np.float32(np.asarray(act1_scale).reshape(-1)[0])
    s_a2 = np.float32(np.asarray(act2_scale).reshape(-1)[0])

    w1_int, s_w1 = _quant_weights(w1)
    w2_int, s_w2 = _quant_weights(w2)
    bq1 = np.clip(np.round(b1 / (s_in * s_w1)), -2.0 ** 31, 2.0 ** 31 - 1).astype(np.float32) * (s_in * s_w1)
    bq2 = np.clip(np.round(b2 / (s_a1 * s_w2)), -2.0 ** 31, 2.0 ** 31 - 1).astype(np.float32) * (s_a1 * s_w2)

    scale1 = float(np.float32(s_w1 / s_a1))
    scale2 = float(np.float32(s_a1 * s_w2 / s_a2))
    out_scale = float(s_a2)
    bias1 = np.ascontiguousarray((bq1 / s_a1).astype(np.float32).reshape(4, 128).T)  # (128, 4)
    bias2 = np.ascontiguousarray((bq2 / s_a2).astype(np.float32).reshape(4, 128).T)

    xp = _phase_planes(x)                                  # (64, 256, 2, 2, 15, 16)
    B, C = xp.shape[:2]
    pl = xp.reshape(B, C, 4, 15, 16)                       # plane = sr*2+sc
    pl_s = np.zeros_like(pl)
    pl_s[..., 0:15] = pl[..., 1:16]                        # shifted 1 col left
    # variant order matches _PVAR / DMA arrival order
    var = np.stack([pl[:, :, 3], pl[:, :, 1], pl_s[:, :, 2],
                    pl_s[:, :, 3], pl_s[:, :, 0], pl_s[:, :, 1]], axis=2)
    var16 = var.astype(np.float16)                         # (B, C, 6, 15, 16)

    w1_l = _w_lhsT(w1_int, 2).astype(np.int8)[:, :, TAP_ORDER]  # tap axis in demand order
    w2_l = _w2_wino(w2_int)                                # fp16 Winograd U

    in_maps = []
    for c in range(N_CORES):
        sl = slice(c * B_PER, (c + 1) * B_PER)
        m = {}
        for name, arr in (("x16", var16[sl]),):
            # (8, 256, 6, 15, 16) -> (ci_p 128, ci_blk 2, v 6, n 8, 15, 16)
            a = arr.transpose(1, 2, 0, 3, 4).reshape(2, 128, 6, B_PER, 15, 16)
            m[name] = np.ascontiguousarray(a.transpose(1, 0, 2, 3, 4, 5))
        m["w1"] = w1_l
        m["w2"] = w2_l
        m["b1"] = bias1
        m["b2"] = bias2
        in_maps.append(m)
    return (scale1, scale2, out_scale), in_maps


def gather_out(results):
    """Per-core (512, 8, 2par, 14, 7t) outputs -> full (64, 512, 14, 14); col = 2t+par."""
    out = np.empty((N_CORES * B_PER, 512, 14, 14), np.float32)
    for c, r in enumerate(results):
        o = np.asarray(r["out"])                           # (co, img, par, row, t)
        a = o.transpose(1, 0, 3, 4, 2).reshape(B_PER, 512, 14, 14)
        out[c * B_PER:(c + 1) * B_PER] = a
    return out


_cache = {}


def kernel(x, w1, b1, w2, b2, in_scale, act1_scale, act2_scale):
    imms, in_maps = prepare(x, w1, b1, w2, b2, in_scale, act1_scale, act2_scale)
    if imms not in _cache:
        _cache[imms] = build_program(*imms)
    nc = _cache[imms]
    res = run_bass_kernel_spmd(nc, in_maps, list(range(N_CORES)))
    return gather_out(res.results)



# revision 39
# speedup vs baseline: 1.0804x; 1.0572x over previous
"""Trainium2 Bass kernel for the quantized BasicBlock (conv3x3/s2 + fakequant + conv3x3/s1 + fakequant).

Sharding: data-parallel over batch across 8 cores (8 images each), weights replicated.

Device math (per core, B=8):
  conv1: implicit GEMM, 9 taps x 2 ci-blocks, single-pass fp16 (x rounded to fp16,
         ~11-bit mantissa; the act1 integer rounding absorbs the error well within
         the 2e-2 gate), integer-valued fp16 weights, fp32 PSUM accum.
  act1:  v = P1*(s_w1/s_a1) + bq1/s_a1; y = clip(rne(v), -128, 127) via the fp32
         magic-number trick on the DVE; y stored as integer-valued bf16 into a
         zero-padded [16x16] layout for conv2.
  conv2: 1-D Winograd F(2,3) along the column axis: V = B^T d (exact ints in
         bf16, 4 DVE ops per act chunk), U = G w (exact half-ints in fp16, host),
         GEMM over (ky, m) in fp32 PSUM (exact), inverse A^T on the DVE (exact
         ints) -> 1.5x fewer PE rows than direct.
  act2:  v2 = P2*(s_a1*s_w2/s_a2) + bq2/s_a2; out = clip(rne(v2), -128, 127) * s_a2.

Stride-2 conv1 is handled by a host-side phase split: x is scattered into 2x2 parity
planes zero-padded to 15x16 rows (32B-aligned rows); 6 plane variants (normal +
one-column-shifted) make every tap window start 4-byte aligned — misaligned rhs
windows cost ~15% per matmul on TRN2.

Input DMAs are chunked in first-use order on the Sync HWDGE queue while ~58 junk
warm-up matmuls cover the load latency and ramp the PE clock; outputs stream on
the Scalar queue.
"""
import os
import sys
from contextlib import ExitStack

import numpy as np
import ml_dtypes

for _p in ("/opt/trn_rl_repo",):
    if _p not in sys.path and os.path.isdir(_p):
        sys.path.insert(0, _p)

import concourse.bacc as bacc
import concourse.tile as tile
import concourse.mybir as mybir
from concourse.bass_utils import run_bass_kernel_spmd

BF16 = ml_dtypes.bfloat16
N_CORES = 8
B_PER = 8           # images per core
MAGIC = float(np.float32(1.5 * 2 ** 23))   # fp32 RNE rounding magic
Alu = mybir.AluOpType
dt = mybir.dt

# tap index k in {0,1,2} -> (parity s, window start offset) for the phase planes
_TAP = {0: (1, 0), 1: (0, 1), 2: (1, 1)}

# cb0 tap order, chosen so plane-variant demand follows DMA arrival order
TAP_ORDER = [0, 6, 3, 1, 7, 2, 8, 4, 5]
# (plane, col_offset) -> x_d variant index; 's' variants are pre-shifted one
# column left on the host so every window starts at column 0 (4B-aligned)
_PVAR = {(3, 0): 0, (1, 0): 1, (2, 1): 2, (3, 1): 3, (0, 1): 4, (1, 1): 5}
# w1's tap axis is permuted into TAP_ORDER on the host, so device-side w1
# indexing uses the order position
_TAP_POS = {t9: o for o, t9 in enumerate(TAP_ORDER)}


def _phase_planes(x):
    """(B, C, 28, 28) f32 -> (B, C, 2, 2, 15, 16): plane[sr][sc][q+1][p+1] = x[2q+sr][2p+sc].

    Rows are padded to 16 so SBUF row stride is 32 B (aligned); col 15 is
    never read by any tap window."""
    B, C = x.shape[:2]
    out = np.zeros((B, C, 2, 2, 15, 16), np.float32)
    for sr in (0, 1):
        for sc in (0, 1):
            out[:, :, sr, sc, 1:15, 1:15] = x[:, :, sr::2, sc::2]
    return out


def _quant_weights(w):
    """Per-tensor int8 narrow-range fake quant; returns (int-valued f32 weights, scale)."""
    s = np.float32(np.max(np.abs(w))) / np.float32(127.0)
    wq = np.clip(np.round(w / s), -127, 127).astype(np.float32)
    return wq, s


def _w2_wino(w_int):
    """(512co, 512ci, 3, 3) ints -> (ci_blk 4, 128, ky 3, m 4, cb 4, co 128) fp16.

    1-D Winograd F(2,3) weight transform along kx: U0 = g0, U1 = (g0+g1+g2)/2,
    U2 = (g0-g1+g2)/2, U3 = g2 — half-integers <= 190.5, exact in fp16."""
    g0 = w_int[..., 0]
    g1 = w_int[..., 1]
    g2 = w_int[..., 2]                                    # (co, ci, ky)
    U = np.stack([g0, (g0 + g1 + g2) * 0.5, (g0 - g1 + g2) * 0.5, g2], axis=0)
    t = U.transpose(2, 3, 0, 1)                           # (ci, ky, m, co)
    t = t.reshape(4, 128, 3, 4, 4, 128)                   # (ci_blk, ci, ky, m, cb, co)
    return np.ascontiguousarray(t).astype(np.float16)


def _w_lhsT(w_int, n_ci_blk):
    """(Cout=512, Cin, 3, 3) int-valued -> (ci_blk, 128, 9, 4, 128) bf16 stationary layout."""
    t = w_int.transpose(2, 3, 1, 0)                      # (3, 3, Cin, 512)
    t = t.reshape(9, n_ci_blk, 128, 4, 128)              # (tap, ci_blk, ci_p, co_blk, co)
    return np.ascontiguousarray(t.transpose(1, 2, 0, 3, 4)).astype(BF16)


_skip_ldw = [False]
_orig_InstMatmult = mybir.InstMatmult


def _patched_InstMatmult(*a, **kw):
    if _skip_ldw[0]:
        kw.setdefault("ldweights", False)
    return _orig_InstMatmult(*a, **kw)


def build_program(scale1, scale2, out_scale):
    """Build the (per-core SPMD) Bass program with the given fp32 immediates."""
    nc = bacc.Bacc("TRN2", target_bir_lowering=False, debug=False,
                   num_devices=N_CORES)

    mybir.InstMatmult = _patched_InstMatmult
    try:
        return _build_body(nc, scale1, scale2, out_scale)
    finally:
        mybir.InstMatmult = _orig_InstMatmult


def _build_body(nc, scale1, scale2, out_scale):
    NT = 4

    # 6 plane variants (normal / col-shifted) so every conv1 tap window starts
    # 4-byte aligned; order = DMA arrival order = cb0 tap demand order.
    x_d = nc.dram_tensor("x16", (128, 2, 6, B_PER, 15, 16), dt.float16, kind="ExternalInput")
    w1_d = nc.dram_tensor("w1", (2, 128, 9, 4, 128), dt.int8, kind="ExternalInput")
    w2_d = nc.dram_tensor("w2", (4, 128, 3, 4, 4, 128), dt.float16, kind="ExternalInput")
    b1_d = nc.dram_tensor("b1", (128, 4), dt.float32, kind="ExternalInput")
    b2_d = nc.dram_tensor("b2", (128, 4), dt.float32, kind="ExternalInput")
    out_d = nc.dram_tensor("out", (512, B_PER, 2, 14, 7), dt.float32, kind="ExternalOutput")

    def mm(out_ap, w_ap, rhs, start, stop, reuse):
        # reuse=True -> PE keeps the already-loaded stationary weights
        _skip_ldw[0] = reuse
        try:
            nc.tensor.matmul(out_ap, w_ap, rhs, start=start, stop=stop)
        finally:
            _skip_ldw[0] = False

    with tile.TileContext(nc) as tc, ExitStack() as ctx:
        const = ctx.enter_context(tc.tile_pool(name="const", bufs=1))
        psum = ctx.enter_context(tc.tile_pool(name="psum", bufs=8, space="PSUM"))
        tmp = ctx.enter_context(tc.tile_pool(name="tmp", bufs=2))
        outp = ctx.enter_context(tc.tile_pool(name="outp", bufs=2))

        # --- SBUF allocations: one tile per DMA chunk for fine-grained deps ---
        # x plane variants: [v][b] -> [128, n, 15, 16]
        x_t = [const.tile([128, 2, B_PER, 15, 16], dt.float16, tag=f"xh{v}", name=f"xh{v}")
               for v in range(6)]
        # w1: [b] -> [128, tap, co_blk, co]; weights arrive int8, DVE converts
        # them to fp16 (values are integers in [-127,127], exact either way)
        w1_t = [const.tile([128, 9, 4, 128], dt.float16, tag=f"w1{b}", name=f"w1t{b}") for b in range(2)]
        w2_t = [const.tile([128, 3, 4, 4, 128], dt.float16, tag=f"w2{b}", name=f"w2t{b}") for b in range(4)]
        w1i_t = [const.tile([128, 9, 4, 128], dt.int8, tag=f"w1i{b}", name=f"w1i{b}") for b in range(2)]
        b1_t = const.tile([128, 4], dt.float32, tag="b1")
        b2_t = const.tile([128, 4], dt.float32, tag="b2")
        act_t = const.tile([128, 4, B_PER, 16, 16], dt.bfloat16, tag="act")  # padded act1
        # Winograd-domain act: V[m][cb][img][row][tile], tile dim packed at 7
        # (row stride 7): a conv2 window's 14 rows x 7 tiles then form one
        # contiguous 98-element run after AP dim-merging, which amortizes the
        # per-run fetch penalty that a padded-to-8 layout pays on every row
        v_t = const.tile([128, 4, 4, B_PER, 16, 7], dt.bfloat16, tag="vt")
        wz = const.tile([128, 256], dt.bfloat16, tag="wz")

        # PE warm-up source zeros; act pad memsets go on the DVE *after* the
        # w1 casts (emitted below) — gpsimd memsets are slow and their SBUF
        # traffic stalls the startup casts, while the DVE is idle from cast
        # end (~20us) until the first act1 epilogue (~29us).
        nc.vector.memset(wz[:], 0.0)

        # --- input loads in first-use order on the Sync HWDGE queue (the two
        # HWDGE queues share HBM bandwidth, so splitting input across both
        # gains nothing; outputs use the Scalar queue) ---
        def load(dst, src):
            nc.sync.dma_start(out=dst, in_=src)

        def load_plane(v):
            load(x_t[v][:], x_d[:, :, v])

        # w1 first (small); tap axis is TAP_ORDER-permuted on host and the
        # int8->fp16 casts are chunked per (tap, ci_blk) in demand order
        for b in range(2):
            load(w1i_t[b][:], w1_d[b])
        for o in range(9):
            for b in range(2):
                nc.vector.tensor_copy(w1_t[b][:, o], w1i_t[b][:, o])
        nc.vector.memset(act_t[:], 0.0)
        load_plane(0)
        load(b1_t[:], b1_d[:])
        for v in range(1, 6):
            load_plane(v)
        for b in range(4):
            load(w2_t[b][:], w2_d[b])
        load(b2_t[:], b2_d[:])

        def quant_chain(dst, src, sc, bias_ap, width=392):
            """dst = clip(rne(src*sc + bias), -128, 127) on the DVE (3 fused ops)."""
            tt = tmp.tile([128, width], dt.float32, tag=f"tt{width}", name="tt")
            nc.vector.tensor_scalar(tt[:], src, sc, bias_ap, op0=Alu.mult, op1=Alu.add)
            nc.vector.tensor_scalar(tt[:], tt[:], MAGIC, MAGIC + 127.0, op0=Alu.add, op1=Alu.min)
            nc.vector.tensor_scalar(dst, tt[:], MAGIC - 128.0, -MAGIC, op0=Alu.max, op1=Alu.add)
            return tt

        # PE warm-up: junk matmuls on the zeroed tile during the input-DMA wait
        # so the HAM clock gate is at full rate when the real stream starts.
        wps = psum.tile([128, 512], dt.float32, tag="ps", name="warmps")
        for i in range(58):
            nc.tensor.matmul(wps[:, 0:256], wz[:, 0:128], wz[:, 0:256],
                             start=True, stop=True)

        # --- conv1 + act1 ---
        # cb0 is tap-major: plane demand spread over the whole 144-MM group to
        # match DMA delivery. cb1-3 are nt-major: each psum bank finishes early
        # and its epilogue overlaps the remaining banks' matmuls.
        def conv1_group(cb, t9, b, ps_list, nts):
            # one stationary weight (t9, b, cb) serving len(nts) matmuls;
            # only the first self-loads the PE array
            ky, kx = divmod(t9, 3)
            sr, r0 = _TAP[ky]
            sc_, c0 = _TAP[kx]
            v = _PVAR[(sr * 2 + sc_, c0)]
            w_ap = w1_t[b][:, _TAP_POS[t9], cb, :]
            for i, nt in enumerate(nts):
                rhs = x_t[v][:, b, 2 * nt:2 * nt + 2, r0:r0 + 14, 0:14]
                mm(ps_list[i][:, 0:392], w_ap, rhs,
                   start=(t9 == TAP_ORDER[0] and b == 0),
                   stop=(t9 == TAP_ORDER[-1] and b == 1),
                   reuse=i > 0)

        def act1_chunk(cb, nt, ps):
            quant_chain(act_t[:, cb, 2 * nt:2 * nt + 2, 1:15, 1:15],
                        ps[:, 0:392], scale1, b1_t[:, cb:cb + 1])
            # 1-D Winograd data transform V = B^T d over the column axis:
            # V0 = d0-d2, V1 = d1+d2, V2 = d2-d1, V3 = d1-d3 (per 4-col tile,
            # stride 2; all 16 rows incl. pads; exact small ints in bf16)
            a = act_t[:, cb, 2 * nt:2 * nt + 2, :, :]
            ev0 = a[:, :, :, 0:13:2]
            od1 = a[:, :, :, 1:14:2]
            ev2 = a[:, :, :, 2:15:2]
            od3 = a[:, :, :, 3:16:2]
            dst = lambda m: v_t[:, m, cb, 2 * nt:2 * nt + 2, :, 0:7]
            nc.vector.tensor_tensor(dst(0), ev0, ev2, op=Alu.subtract)
            nc.vector.tensor_tensor(dst(1), od1, ev2, op=Alu.add)
            nc.vector.tensor_tensor(dst(2), ev2, od1, op=Alu.subtract)
            nc.vector.tensor_tensor(dst(3), od1, od3, op=Alu.subtract)

        for cb in range(4):
            if cb == 0:
                # tap-major: plane demand spread over the whole group to match
                # the DMA delivery ramp; 8 matmuls per weight load
                ps_n = [psum.tile([128, 512], dt.float32, tag="ps", name="ps")
                        for _ in range(NT)]
                for t9 in TAP_ORDER:
                    for b in range(2):
                        conv1_group(cb, t9, b, ps_n, range(NT))
                for nt in range(NT):
                    act1_chunk(cb, nt, ps_n[nt])
            else:
                # nt-pair-major: each bank pair finishes at half-time so its
                # epilogue overlaps the rest; the last cb runs single-nt
                # groups so its final epilogue chain is short (conv2's first
                # taps wait on it)
                halves = ([[0, 1], [2, 3]] if cb < 3 else [[0], [1], [2], [3]])
                for nts in halves:
                    ps_p = [psum.tile([128, 512], dt.float32, tag="ps", name="ps")
                            for _ in nts]
                    for t9 in TAP_ORDER:
                        for b in range(2):
                            conv1_group(cb, t9, b, ps_p, nts)
                    for i, nt in enumerate(nts):
                        act1_chunk(cb, nt, ps_p[i])

        # --- conv2 (1-D Winograd) + act2 ---
        # For each (out-cb, image-pair chunk): 8 PSUM banks hold the 4 m-
        # positions x 2 chunks; GEMM accumulates over (ky, ci-blk). The
        # epilogue applies the inverse transform A^T (o0 = m0+m1+m2,
        # o1 = m1-m2-m3, both exact ints in fp32), then the act2 quant chain.
        def conv2_epilogue(cb, i0, ni, psm):
            # psm: list of 4 PSUM tiles [128, ni*98] (m = 0..3) for images
            # i0..i0+ni (ni <= 4). Only one PSUM operand is allowed per DVE
            # op, so m1 is staged to SBUF first.
            w = ni * 98
            tq = tmp.tile([128, 4, 2, 14, 7], dt.float32, tag="tq", name="tq")
            s1 = tmp.tile([128, 392], dt.float32, tag="s1", name="s1")
            ti = tmp.tile([128, 392], dt.float32, tag="ti", name="ti")
            t2 = tmp.tile([128, 392], dt.float32, tag="t2", name="t2")
            nc.vector.tensor_copy(s1[:, 0:w], psm[1][:, 0:w])
            nc.vector.tensor_tensor(ti[:, 0:w], psm[0][:, 0:w], s1[:, 0:w], op=Alu.add)
            nc.vector.tensor_tensor(tq[:, 0:ni, 0], ti[:, 0:w], psm[2][:, 0:w], op=Alu.add)
            nc.vector.tensor_tensor(t2[:, 0:w], s1[:, 0:w], psm[2][:, 0:w], op=Alu.subtract)
            nc.vector.tensor_tensor(tq[:, 0:ni, 1], t2[:, 0:w], psm[3][:, 0:w], op=Alu.subtract)
            ot = outp.tile([128, 784], dt.float32, tag="ot", name="ot")
            tq_dst = tmp.tile([128, 784], dt.float32, tag="tq2", name="tq2")
            quant_chain(tq_dst[:, 0:2 * w], tq[:, 0:ni], scale2, b2_t[:, cb:cb + 1],
                        width=2 * w)
            nc.vector.tensor_scalar_mul(ot[:, 0:2 * w], tq_dst[:, 0:2 * w], out_scale)
            nc.scalar.dma_start(
                out=out_d[cb * 128:(cb + 1) * 128, i0:i0 + ni], in_=ot[:, 0:2 * w])

        def conv2_group(cb, chunks):
            # one group: chunks = list of (img_start, n_imgs); 4 m-banks per
            # chunk; stationary (ky, m, b, cb) reused across the chunks
            ps = {(m, i): psum.tile([128, 512], dt.float32, tag="ps", name="ps")
                  for m in range(4) for i in range(len(chunks))}
            # m outermost: ps[0..2] finish their accumulation 12-36 matmuls
            # before the group ends, so the epilogue's inverse-transform
            # prefix overlaps the tail of the matmul stream
            for m in range(4):
                for ky in range(3):
                    for b in range(4):
                        w_ap = w2_t[b][:, ky, m, cb, :]
                        for i, (i0, ni) in enumerate(chunks):
                            rhs = v_t[:, m, b, i0:i0 + ni, ky:ky + 14, 0:7]
                            mm(ps[(m, i)][:, 0:ni * 98], w_ap, rhs,
                               start=(ky == 0 and b == 0),
                               stop=(ky == 2 and b == 3),
                               reuse=i > 0)
            for i, (i0, ni) in enumerate(chunks):
                conv2_epilogue(cb, i0, ni, [ps[(m, i)] for m in range(4)])

        for cb in range(4):
            if cb < 3:
                conv2_group(cb, [(0, 4), (4, 4)])
            else:
                # last cb: staggered, shrinking chunks so the final exposed
                # epilogue is a single image
                conv2_group(cb, [(0, 4)])
                conv2_group(cb, [(4, 2)])
                conv2_group(cb, [(6, 1)])
                conv2_group(cb, [(7, 1)])

    _dedupe_ldweights(nc)
    nc.compile()
    return nc


def _dedupe_ldweights(nc):
    """Drop LDWEIGHTS whose stationary operand is identical to the previous
    one on the PE stream (only MATMULs in between): the PE array keeps its
    loaded weights, so consecutive same-weight matmuls need a single load."""
    def sig_of(inst):
        a0 = inst.ins[0]
        try:
            return (a0.memref, a0.offset, str(a0.ap), str(a0.dtype))
        except Exception:
            return None

    removed = 0
    for blk in nc.main_func.blocks:
        last = None
        keep = []
        for inst in blk.instructions:
            tn = type(inst).__name__
            if inst.engine == mybir.EngineType.PE:
                if tn == "InstLdweights":
                    sig = sig_of(inst)
                    si = inst.sync_info
                    clean = si is None or (not si.on_wait and not si.on_update)
                    if sig is not None and sig == last and clean:
                        removed += 1
                        continue
                    last = sig
                elif tn != "InstMatmult":
                    last = None
            keep.append(inst)
        blk.instructions[:] = keep
    return removed


def prepare(x, w1, b1, w2, b2, in_scale, act1_scale, act2_scale):
    """Host-side prep: quantize weights, build per-core input maps + immediates."""
    x = np.asarray(x, np.float32)
    w1 = np.asarray(w1, np.float32)
    b1 = np.asarray(b1, np.float32)
    w2 = np.asarray(w2, np.float32)
    b2 = np.asarray(b2, np.float32)
    s_in = np.float32(np.asarray(in_scale).reshape(-1)[0])
    s_a1 = np.float32(np.asarray(act1_scale).reshape(-1)[0])
    s_a2 = np.float32(np.asarray(act2_scale).reshape(-1)[0])

    w1_int, s_w1 = _quant_weights(w1)
    w2_int, s_w2 = _quant_weights(w2)
    bq1 = np.clip(np.round(b1 / (s_in * s_w1)), -2.0 ** 31, 2.0 ** 31 - 1).astype(np.float32) * (s_in * s_w1)
    bq2 = np.clip(np.round(b2 / (s_a1 * s_w2)), -2.0 ** 31, 2.0 ** 31 - 1).astype(np.float32) * (s_a1 * s_w2)

    scale1 = float(np.float32(s_w1 / s_a1))
    scale2 = float(np.float32(s_a1 * s_w2 / s_a2))
    out_scale = float(s_a2)
    bias1 = np.ascontiguousarray((bq1 / s_a1).astype(np.float32).reshape(4, 128).T)  # (128, 4)
    bias2 = np.ascontiguousarray((bq2 / s_a2).astype(np.float32).reshape(4, 128).T)

    xp = _phase_planes(x)                                  # (64, 256, 2, 2, 15, 16)
    B, C = xp.shape[:2]
    pl = xp.reshape(B, C, 4, 15, 16)                       # plane = sr*2+sc
    pl_s = np.zeros_like(pl)
    pl_s[..., 0:15] = pl[..., 1:16]                        # shifted 1 col left
    # variant order matches _PVAR / DMA arrival order
    var = np.stack([pl[:, :, 3], pl[:, :, 1], pl_s[:, :, 2],
                    pl_s[:, :, 3], pl_s[:, :, 0], pl_s[:, :, 1]], axis=2)
    var16 = var.astype(np.float16)                         # (B, C, 6, 15, 16)

    w1_l = _w_lhsT(w1_int, 2).astype(np.int8)[:, :, TAP_ORDER]  # tap axis in demand order
    w2_l = _w2_wino(w2_int)                                # fp16 Winograd U

    in_maps = []
    for c in range(N_CORES):
        sl = slice(c * B_PER, (c + 1) * B_PER)
        m = {}
        for name, arr in (("x16", var16[sl]),):
            # (8, 256, 6, 15, 16) -> (ci_p 128, ci_blk 2, v 6, n 8, 15, 16)
            a = arr.transpose(1, 2, 0, 3, 4).reshape(2, 128, 6, B_PER, 15, 16)
            m[name] = np.ascontiguousarray(a.transpose(1, 0, 2, 3, 4, 5))
        m["w1"] = w1_l
        m["w2"] = w2_l
        m["b1"] = bias1
        m["b2"] = bias2
        in_maps.append(m)
    return (scale1, scale2, out_scale), in_maps


def gather_out(results):
    """Per-core (512, 8, 2par, 14, 7t) outputs -> full (64, 512, 14, 14); col = 2t+par."""
    out = np.empty((N_CORES * B_PER, 512, 14, 14), np.float32)
    for c, r in enumerate(results):
        o = np.asarray(r["out"])                           # (co, img, par, row, t)
        a = o.transpose(1, 0, 3, 4, 2).reshape(B_PER, 512, 14, 14)
        out[c * B_PER:(c + 1) * B_PER] = a
    return out


_cache = {}


def kernel(x, w1, b1, w2, b2, in_scale, act1_scale, act2_scale):
    imms, in_maps = prepare(x, w1, b1, w2, b2, in_scale, act1_scale, act2_scale)
    if imms not in _cache:
        _cache[imms] = build_program(*imms)
    nc = _cache[imms]
    res = run_bass_kernel_spmd(nc, in_maps, list(range(N_CORES)))
    return gather_out(res.results)

